# revision 15
# baseline (speedup 1.0000x reference)
"""BinaryTreeGRU Trainium2 kernel (v2: all-bf16 datapath).

Batch of B=64 complete binary trees (L=512 leaves, 1023 nodes each),
data-parallel over trees across 8 NeuronCores (8 trees/core).

Layout: feature-major: every activation tensor lives in SBUF as
[128 partitions, 2 feature-blocks, n_nodes] (mem dim 256 = 2 blocks of 128).
Level l has N_l = 8 * 512 / 2^l node-columns per core. h for each level is
stored deinterleaved ("parity" layout): column 2j+p lives at [cb, p, j], so
the NEXT level's children are two contiguous halves (h_l / h_r views) and
only the single h-write per node is strided.

All activations/gates/h are bf16 (DVE 2x packed mode, half DMA); matmul
accumulation stays fp32 in PSUM. Weights bf16 (FWL); leaf x stays f32r.

Per internal level:
  PE : rzh = Wrzh @ [h_l ; h_r]   (32 MMs/chunk)  +  Wgh @ s (4 MMs)
  ACT: gates = sigmoid(rzh + 1)   g = tanh(Wgh s)
  DVE: P4 = r4*h4, s = P4l+P4r, Z4 = z4*h4, zs = zl+zr,
       tt = 1 - zs/2 (tensor_scalar), tg = tt*g
  GPS: zh = Z4l+Z4r, h = tg + zh (strided parity write)

Host side only reshapes/casts numpy arrays for sharding and gathers.
"""

import os
from contextlib import ExitStack

import ml_dtypes
import numpy as np

import concourse.bass as bass
import concourse.mybir as mybir
import concourse.tile as tile
from concourse import bacc
from concourse.bass_utils import run_bass_kernel_spmd

F32 = mybir.dt.float32
F32R = mybir.dt.float32r
BF16 = mybir.dt.bfloat16
MULT = mybir.AluOpType.mult
ADD = mybir.AluOpType.add
SIGMOID = mybir.ActivationFunctionType.Sigmoid
TANH = mybir.ActivationFunctionType.Tanh
COPY = mybir.ActivationFunctionType.Copy

MEM = 256
IN_DIM = 256
B = 64
L = 512
NCORES = 8
BLOC = B // NCORES            # trees per core
N0 = BLOC * L                 # leaf columns per core = 4096
NLEVELS = 10                  # 4096,2048,...,8 columns
NCOLS = [N0 >> l for l in range(NLEVELS)]
TOT = sum(NCOLS)              # 8184
OFFS = np.cumsum([0] + NCOLS).tolist()
NC = 512                      # node-column chunk

LAST_RESULT = {}


def _wavefront_order(nchunks, d=2):
    """Topological chunk order interleaving levels.

    Chunk (lv, ci)'s parents are (lv-1, 2ci) and (lv-1, 2ci+1) when level
    lv-1 has 2x the chunks; when levels shrink below NC the parent is the
    single previous-level chunk. Child front must come >= parent_pos + d
    (parent back emitted d steps after its front). Returns list of (lv, ci).
    """
    pos = {}
    order = []
    remaining = [(lv, ci) for lv in range(NLEVELS)
                 for ci in range(nchunks[lv])]

    def parents(lv, ci):
        if lv == 0:
            return []
        if nchunks[lv - 1] == 2 * nchunks[lv]:
            return [(lv - 1, 2 * ci), (lv - 1, 2 * ci + 1)]
        return [(lv - 1, pc) for pc in range(nchunks[lv - 1])]

    t = 0
    while remaining:
        ready = []
        for (lv, ci) in remaining:
            ps = parents(lv, ci)
            dd = d if lv > 1 else (d if lv == 1 else 0)
            if all(p in pos and pos[p] + (2 if lv == 1 else dd + 1) <= t
                   for p in ps):
                ready.append((lv, ci))
        if ready:
            ch = max(ready, key=lambda c: (c[0], -c[1]))
            pos[ch] = t
            order.append(ch)
            remaining.remove(ch)
        else:
            order.append(None)   # spacing step (emit only a back)
        t += 1
    return order


def build_nc(fast_bias: bool):
    nc = bacc.Bacc("TRN2", target_bir_lowering=False, debug=False)

    d_x = nc.dram_tensor("xT", [2, 128, N0], BF16, kind="ExternalInput")
    d_wrzh = nc.dram_tensor("wrzh", [4, 128, 1024], BF16, kind="ExternalInput")
    d_wgrzx = nc.dram_tensor("wgrzx", [2, 128, 768], BF16, kind="ExternalInput")
    d_wgh = nc.dram_tensor("wgh", [2, 128, 256], BF16, kind="ExternalInput")
    d_bias = nc.dram_tensor("bias6", [6, 128, 1], F32, kind="ExternalInput")
    d_out = nc.dram_tensor("out", [2, 128, TOT], BF16, kind="ExternalOutput")

    x = d_x.ap()
    wrzh = d_wrzh.ap()
    wgrzx = d_wgrzx.ap()
    wgh = d_wgh.ap()
    bias6 = d_bias.ap()
    out = d_out.ap()

    mm = nc.tensor.matmul
    nchunks = [max(1, NCOLS[lv] // NC) for lv in range(NLEVELS)]

    with tile.TileContext(nc) as tc, ExitStack() as ctx:
        singles = ctx.enter_context(tc.tile_pool(name="singles", bufs=1))
        xpool = ctx.enter_context(tc.tile_pool(name="xpool", bufs=3))
        gates_pool = ctx.enter_context(tc.tile_pool(name="gates", bufs=3))
        gsb_pool = ctx.enter_context(tc.tile_pool(name="gsb", bufs=3))
        scratch = ctx.enter_context(tc.tile_pool(name="scratch", bufs=2))
        psum = ctx.enter_context(tc.tile_pool(name="psum", bufs=4, space="PSUM"))

        # --- load constants ---
        w_rzh = []
        for kc in range(4):
            t = singles.tile([128, 1024], BF16, tag=f"wrzh{kc}", name=f"wrzh{kc}")
            nc.sync.dma_start(out=t, in_=wrzh[kc])
            w_rzh.append(t)
        w_grzx = []
        for kc in range(2):
            t = singles.tile([128, 768], BF16, tag=f"wgrzx{kc}", name=f"wgrzx{kc}")
            nc.sync.dma_start(out=t, in_=wgrzx[kc])
            w_grzx.append(t)
        w_gh = []
        for kc in range(2):
            t = singles.tile([128, 256], BF16, tag=f"wgh{kc}", name=f"wgh{kc}")
            nc.sync.dma_start(out=t, in_=wgh[kc])
            w_gh.append(t)
        b_t = []
        for i in range(6):
            t = singles.tile([128, 1], F32, tag=f"b{i}", name=f"b{i}")
            nc.sync.dma_start(out=t, in_=bias6[i])
            b_t.append(t)
        # b_t: [0]=bg0 [1]=bg1 [2]=bA0 [3]=bA1 [4]=bB0 [5]=bB1

        h_t = [singles.tile([128, 2, 2, max(1, NCOLS[l] // 2)], BF16,
                            tag=f"h{l}", name=f"h{l}", bufs=1)
               for l in range(NLEVELS)]

        def h_scatter(lv, c0, ncur):
            """Column-ordered [128, 2, ncur] view of h_t[lv] (parity layout)
            covering columns c0..c0+ncur: dims (cb, j, par) with par stride."""
            t = h_t[lv]
            half = max(1, NCOLS[lv] // 2)
            j0 = c0 // 2
            n2 = ncur // 2
            return bass.AP(tensor=t.tensor, offset=t.offset + j0,
                           ap=[list(t.ap[0]), [2 * half, 2], [1, n2],
                               [half, 2]])

        def h4_view(lv, c0, ncur):
            """[128, 2(lr), 2(cb), ncur] view of h_t[lv] matching the
            (lr-major, cb-minor) gate-block ordering [xl0 xl1 xr0 xr1]."""
            t = h_t[lv]
            half = max(1, NCOLS[lv] // 2)
            return bass.AP(tensor=t.tensor, offset=t.offset + c0,
                           ap=[list(t.ap[0]), [half, 2], [2 * half, 2],
                               [1, ncur]])

        state = {}   # (lv, ci) -> dict of tiles/views for the back phase

        def emit_leaf_front(ci):
            c0 = ci * NC
            x_c = []
            for kc in range(2):
                t = xpool.tile([128, NC], BF16, tag=f"x{kc}", name=f"x{kc}")
                nc.sync.dma_start(out=t, in_=x[kc, :, c0:c0 + NC])
                x_c.append(t)
            srzx = gates_pool.tile([128, 4, NC], BF16, tag="gr", name="srzx")
            for q in range(2):
                ps = psum.tile([128, 2, NC], F32, tag="ps", name="ps_rzx")
                for mb in range(2):
                    col = 256 + (q * 2 + mb) * 128
                    for kc in range(2):
                        mm(ps[:, mb, :], w_grzx[kc][:, col:col + 128],
                           x_c[kc], start=(kc == 0), stop=(kc == 1))
                if fast_bias:
                    nc.scalar.activation(srzx[:, 2 * q:2 * q + 2, :], ps,
                                         SIGMOID, bias=1.0)
                else:
                    for mb in range(2):
                        nc.scalar.activation(
                            srzx[:, 2 * q + mb, :], ps[:, mb, :],
                            SIGMOID, bias=b_t[2 + 2 * q + mb])
            ps_gx = psum.tile([128, 2, NC], F32, tag="ps", name="ps_gx")
            for mb in range(2):
                for kc in range(2):
                    mm(ps_gx[:, mb, :], w_grzx[kc][:, 128 * mb:128 * mb + 128],
                       x_c[kc], start=(kc == 0), stop=(kc == 1))
            tg = gsb_pool.tile([128, 2, NC], BF16, tag="gsb", name="tg")
            if fast_bias:
                nc.scalar.activation(tg, ps_gx, TANH, bias=0.0)
            else:
                for mb in range(2):
                    nc.scalar.activation(tg[:, mb, :], ps_gx[:, mb, :],
                                         TANH, bias=b_t[mb])
            zsum = scratch.tile([128, 2, NC], BF16, tag="sa", name="zsum")
            nc.gpsimd.tensor_add(zsum, srzx[:, 0:2, :], srzx[:, 2:4, :])
            tt = scratch.tile([128, 2, NC], BF16, tag="sb", name="tt")
            nc.vector.tensor_scalar(tt, zsum, -0.5, 1.0, MULT, ADD)
            nc.vector.tensor_mul(h_scatter(0, c0, NC), tt, tg)
            if ci == nchunks[0] - 1:
                for cb in range(2):
                    nc.sync.dma_start(out=out[cb, :, OFFS[0]:OFFS[1]],
                                      in_=h_t[0][:, cb, :, :])

        def emit_front(lv, ci):
            if lv == 0:
                emit_leaf_front(ci)
                return
            n = NCOLS[lv]
            ncur = min(n, NC)
            c0 = ci * ncur
            h4 = h4_view(lv - 1, c0, ncur)

            # gr blocks [rl0, rl1, rr0, rr1]; gz blocks [zl0, zl1, zr0, zr1]
            gr = gates_pool.tile([128, 4, ncur], BF16, tag="gr", name="gr")
            gz = gates_pool.tile([128, 4, ncur], BF16, tag="gz", name="gz")
            hp = h_t[lv - 1]
            for q in range(4):
                dst = (gr, gz)[q // 2]
                half = (q % 2) * 2
                ps = psum.tile([128, 2, ncur], F32, tag="ps", name="ps_rz")
                for mb in range(2):
                    col = (q * 2 + mb) * 128
                    for kc in range(4):
                        mm(ps[:, mb, :], w_rzh[kc][:, col:col + 128],
                           hp[:, kc % 2, kc // 2, c0:c0 + ncur],
                           start=(kc == 0), stop=(kc == 3))
                if fast_bias:
                    nc.scalar.activation(dst[:, half:half + 2, :], ps,
                                         SIGMOID, bias=1.0)
                else:
                    bi = (2, 4, 2, 4)[q]
                    for mb in range(2):
                        nc.scalar.activation(dst[:, half + mb, :],
                                             ps[:, mb, :],
                                             SIGMOID, bias=b_t[bi + mb])

            # r-path: s = r_l*h_l + r_r*h_r
            p4 = scratch.tile([128, 4, ncur], BF16, tag="sa", name="p4")
            s = scratch.tile([128, 2, ncur], BF16, tag="sc", name="s", bufs=2)
            nc.vector.tensor_mul(p4, gr, h4)
            nc.vector.tensor_add(s, p4[:, 0:2, :], p4[:, 2:4, :])
            # z-path: zh = z_l*h_l + z_r*h_r ; tt = 1 - (z_l + z_r)/2
            z4 = scratch.tile([128, 4, ncur], BF16, tag="sb", name="z4")
            zh = scratch.tile([128, 2, ncur], BF16, tag="sd", name="zh", bufs=2)
            nc.vector.tensor_mul(z4, gz, h4)
            e1 = nc.gpsimd if ncur >= 256 else nc.vector
            e1.tensor_add(zh, z4[:, 0:2, :], z4[:, 2:4, :])
            zs = scratch.tile([128, 2, ncur], BF16, tag="se", name="zs")
            nc.vector.tensor_add(zs, gz[:, 0:2, :], gz[:, 2:4, :])
            tt = gsb_pool.tile([128, 2, ncur], BF16, tag="tt", name="tt", bufs=2)
            nc.vector.tensor_scalar(tt, zs, -0.5, 1.0, MULT, ADD)
            state[(lv, ci)] = dict(s=s, zh=zh, tt=tt, c0=c0, ncur=ncur)

        def emit_back(lv, ci):
            if lv == 0:
                return
            st = state.pop((lv, ci))
            s, zh, tt = st["s"], st["zh"], st["tt"]
            c0, ncur = st["c0"], st["ncur"]

            psg = psum.tile([128, 2, ncur], F32, tag="ps", name="ps_g")
            for mb in range(2):
                for kc in range(2):
                    mm(psg[:, mb, :], w_gh[kc][:, 128 * mb:128 * mb + 128],
                       s[:, kc, :], start=(kc == 0), stop=(kc == 1))
            g_sb = gsb_pool.tile([128, 2, ncur], BF16, tag="gsb", name="g_sb")
            if fast_bias:
                nc.scalar.activation(g_sb, psg, TANH, bias=0.0)
            else:
                for mb in range(2):
                    nc.scalar.activation(g_sb[:, mb, :], psg[:, mb, :],
                                         TANH, bias=b_t[mb])
            tg = scratch.tile([128, 2, ncur], BF16, tag="sg", name="tg")
            nc.vector.tensor_mul(tg, tt, g_sb)
            nc.vector.tensor_add(h_scatter(lv, c0, ncur), tg, zh)
            if ci == nchunks[lv] - 1:
                for cb in range(2):
                    nc.sync.dma_start(out=out[cb, :, OFFS[lv]:OFFS[lv + 1]],
                                      in_=h_t[lv][:, cb, :])

        D = 3
        order = _wavefront_order(nchunks, D)

        def parent_list(lv, ci):
            if lv == 0:
                return []
            if nchunks[lv - 1] == 2 * nchunks[lv]:
                return [(lv - 1, 2 * ci), (lv - 1, 2 * ci + 1)]
            return [(lv - 1, pc) for pc in range(nchunks[lv - 1])]

        pending = []
        done = set()

        def pop_back():
            b = pending.pop(0)
            emit_back(*b)
            done.add(b)

        for ch in order:
            if ch is None:
                if pending:
                    pop_back()
                continue
            lv, ci = ch
            for par in parent_list(lv, ci):
                while par not in done:
                    pop_back()
            emit_front(lv, ci)
            pending.append(ch)
            while len(pending) > D:
                pop_back()
        while pending:
            pop_back()

    nc.compile()
    return nc


def _prep_inputs(inputs, Wgrzx, bgrzx, Wrzh, Wgh):
    """Host-side shard + layout prep. Returns (in_maps, fast_bias)."""
    x = np.ascontiguousarray(inputs, dtype=np.float32)
    Wgrzx = np.asarray(Wgrzx, dtype=np.float32)
    bgrzx = np.asarray(bgrzx, dtype=np.float32)
    Wrzh = np.asarray(Wrzh, dtype=np.float32)
    Wgh = np.asarray(Wgh, dtype=np.float32)

    fast_bias = bool(
        np.all(bgrzx[:MEM] == 0.0) and np.all(bgrzx[MEM:] == 1.0))

    wgrzxT = np.ascontiguousarray(
        Wgrzx.T.reshape(2, 128, 768)).astype(ml_dtypes.bfloat16)
    wrzhT = np.ascontiguousarray(
        Wrzh.T.reshape(4, 128, 1024)).astype(ml_dtypes.bfloat16)
    wghT = np.ascontiguousarray(
        Wgh.T.reshape(2, 128, 256)).astype(ml_dtypes.bfloat16)
    bias6 = np.ascontiguousarray(bgrzx.reshape(6, 128, 1))

    in_maps = []
    for c in range(NCORES):
        xc = x[c * BLOC:(c + 1) * BLOC].reshape(N0, IN_DIM)
        xT = np.ascontiguousarray(xc.T).reshape(2, 128, N0).astype(
            ml_dtypes.bfloat16)
        in_maps.append({
            "xT": xT,
            "wrzh": wrzhT,
            "wgrzx": wgrzxT,
            "wgh": wghT,
            "bias6": bias6,
        })
    return in_maps, fast_bias


def _gather(results):
    """results: list of per-core {'out': [2,128,TOT] bf16} -> [B,2L-1,MEM]."""
    outs = []
    for c in range(len(results)):
        fm = np.asarray(results[c]["out"]).astype(np.float32).reshape(MEM, TOT)
        levels = []
        for lv in range(NLEVELS):
            n = NCOLS[lv]
            blk = fm[:, OFFS[lv]:OFFS[lv + 1]]
            nat = np.empty_like(blk)
            nat[:, 0::2] = blk[:, :n // 2]
            nat[:, 1::2] = blk[:, n // 2:]
            k = n // BLOC
            levels.append(nat.reshape(MEM, BLOC, k).transpose(1, 2, 0))
        outs.append(np.concatenate(levels, axis=1))
    return np.ascontiguousarray(
        np.concatenate(outs, axis=0), dtype=np.float32)


def kernel(**inputs):
    in_maps, fast_bias = _prep_inputs(
        inputs["inputs"], inputs["Wgrzx"], inputs["bgrzx"],
        inputs["Wrzh"], inputs["Wgh"])
    nc = build_nc(fast_bias)
    trace = bool(int(os.environ.get("BTGRU_TRACE", "0")))
    res = run_bass_kernel_spmd(
        nc, in_maps, core_ids=list(range(NCORES)), trace=trace)
    LAST_RESULT.clear()
    LAST_RESULT["exec_time_ns"] = res.exec_time_ns
    LAST_RESULT["profile_json"] = res.profile_json
    return _gather(res.results)


# revision 21
# speedup vs baseline: 1.0162x; 1.0162x over previous
"""BinaryTreeGRU Trainium2 kernel (v2: all-bf16 datapath).

Batch of B=64 complete binary trees (L=512 leaves, 1023 nodes each),
data-parallel over trees across 8 NeuronCores (8 trees/core).

Layout: feature-major: every activation tensor lives in SBUF as
[128 partitions, 2 feature-blocks, n_nodes] (mem dim 256 = 2 blocks of 128).
Level l has N_l = 8 * 512 / 2^l node-columns per core. h for each level is
stored deinterleaved ("parity" layout): column 2j+p lives at [cb, p, j], so
the NEXT level's children are two contiguous halves (h_l / h_r views) and
only the single h-write per node is strided.

All activations/gates/h are bf16 (DVE 2x packed mode, half DMA); matmul
accumulation stays fp32 in PSUM. Weights bf16 (FWL); leaf x stays f32r.

Per internal level:
  PE : rzh = Wrzh @ [h_l ; h_r]   (32 MMs/chunk)  +  Wgh @ s (4 MMs)
  ACT: gates = sigmoid(rzh + 1)   g = tanh(Wgh s)
  DVE: P4 = r4*h4, s = P4l+P4r, Z4 = z4*h4, zs = zl+zr,
       tt = 1 - zs/2 (tensor_scalar), tg = tt*g
  GPS: zh = Z4l+Z4r, h = tg + zh (strided parity write)

Host side only reshapes/casts numpy arrays for sharding and gathers.
"""

import os
from contextlib import ExitStack

import ml_dtypes
import numpy as np

import concourse.bass as bass
import concourse.mybir as mybir
import concourse.tile as tile
from concourse import bacc
from concourse.bass_utils import run_bass_kernel_spmd

F32 = mybir.dt.float32
F32R = mybir.dt.float32r
BF16 = mybir.dt.bfloat16
MULT = mybir.AluOpType.mult
ADD = mybir.AluOpType.add
SIGMOID = mybir.ActivationFunctionType.Sigmoid
TANH = mybir.ActivationFunctionType.Tanh
COPY = mybir.ActivationFunctionType.Copy

MEM = 256
IN_DIM = 256
B = 64
L = 512
NCORES = 8
BLOC = B // NCORES            # trees per core
N0 = BLOC * L                 # leaf columns per core = 4096
NLEVELS = 10                  # 4096,2048,...,8 columns
NCOLS = [N0 >> l for l in range(NLEVELS)]
TOT = sum(NCOLS)              # 8184
OFFS = np.cumsum([0] + NCOLS).tolist()
NC = 512                      # node-column chunk

LAST_RESULT = {}


def _wavefront_order(nchunks, d=2):
    """Topological chunk order interleaving levels.

    Chunk (lv, ci)'s parents are (lv-1, 2ci) and (lv-1, 2ci+1) when level
    lv-1 has 2x the chunks; when levels shrink below NC the parent is the
    single previous-level chunk. Child front must come >= parent_pos + d
    (parent back emitted d steps after its front). Returns list of (lv, ci).
    """
    pos = {}
    order = []
    remaining = [(lv, ci) for lv in range(NLEVELS)
                 for ci in range(nchunks[lv])]

    def parents(lv, ci):
        if lv == 0:
            return []
        if nchunks[lv - 1] == 2 * nchunks[lv]:
            return [(lv - 1, 2 * ci), (lv - 1, 2 * ci + 1)]
        return [(lv - 1, pc) for pc in range(nchunks[lv - 1])]

    t = 0
    while remaining:
        ready = []
        for (lv, ci) in remaining:
            ps = parents(lv, ci)
            dd = d if lv > 1 else (d if lv == 1 else 0)
            if all(p in pos and pos[p] + (2 if lv == 1 else dd + 1) <= t
                   for p in ps):
                ready.append((lv, ci))
        if ready:
            ch = max(ready, key=lambda c: (c[0], -c[1]))
            pos[ch] = t
            order.append(ch)
            remaining.remove(ch)
        else:
            order.append(None)   # spacing step (emit only a back)
        t += 1
    return order


def build_nc(fast_bias: bool):
    nc = bacc.Bacc("TRN2", target_bir_lowering=False, debug=False)

    d_x = nc.dram_tensor("xT", [2, 128, N0], BF16, kind="ExternalInput")
    d_wrzh = nc.dram_tensor("wrzh", [4, 128, 1024], BF16, kind="ExternalInput")
    d_wgrzx = nc.dram_tensor("wgrzx", [2, 128, 768], BF16, kind="ExternalInput")
    d_wgh = nc.dram_tensor("wgh", [2, 128, 256], BF16, kind="ExternalInput")
    d_bias = nc.dram_tensor("bias6", [6, 128, 1], F32, kind="ExternalInput")
    d_out = nc.dram_tensor("out", [2, 128, TOT], BF16, kind="ExternalOutput")

    x = d_x.ap()
    wrzh = d_wrzh.ap()
    wgrzx = d_wgrzx.ap()
    wgh = d_wgh.ap()
    bias6 = d_bias.ap()
    out = d_out.ap()

    mm = nc.tensor.matmul
    nchunks = [max(1, NCOLS[lv] // NC) for lv in range(NLEVELS)]

    with tile.TileContext(nc) as tc, ExitStack() as ctx:
        singles = ctx.enter_context(tc.tile_pool(name="singles", bufs=1))
        xpool = ctx.enter_context(tc.tile_pool(name="xpool", bufs=3))
        gates_pool = ctx.enter_context(tc.tile_pool(name="gates", bufs=3))
        gsb_pool = ctx.enter_context(tc.tile_pool(name="gsb", bufs=3))
        scratch = ctx.enter_context(tc.tile_pool(name="scratch", bufs=2))
        psum = ctx.enter_context(tc.tile_pool(name="psum", bufs=4, space="PSUM"))

        # --- load constants ---
        w_rzh = []
        for kc in range(4):
            t = singles.tile([128, 1024], BF16, tag=f"wrzh{kc}", name=f"wrzh{kc}")
            nc.sync.dma_start(out=t, in_=wrzh[kc])
            w_rzh.append(t)
        w_grzx = []
        for kc in range(2):
            t = singles.tile([128, 768], BF16, tag=f"wgrzx{kc}", name=f"wgrzx{kc}")
            nc.sync.dma_start(out=t, in_=wgrzx[kc])
            w_grzx.append(t)
        w_gh = []
        for kc in range(2):
            t = singles.tile([128, 256], BF16, tag=f"wgh{kc}", name=f"wgh{kc}")
            nc.sync.dma_start(out=t, in_=wgh[kc])
            w_gh.append(t)
        b_t = []
        for i in range(6):
            t = singles.tile([128, 1], F32, tag=f"b{i}", name=f"b{i}")
            nc.sync.dma_start(out=t, in_=bias6[i])
            b_t.append(t)
        # b_t: [0]=bg0 [1]=bg1 [2]=bA0 [3]=bA1 [4]=bB0 [5]=bB1

        h_t = [singles.tile([128, 2, 2, max(1, NCOLS[l] // 2)], BF16,
                            tag=f"h{l}", name=f"h{l}", bufs=1)
               for l in range(NLEVELS)]

        def parj(t, ncur):
            """[128, 2(cb), 2(par), ncur/2] strided-read view of a column-
            ordered [128, 2, ncur] tile: element (cb,par,j) = t[cb, 2j+par].
            Lets the parity-deinterleaved h write be CONTIGUOUS (strided
            bf16 writes are pathologically slow: read-modify-write)."""
            return bass.AP(tensor=t.tensor, offset=t.offset,
                           ap=[list(t.ap[0]), [ncur, 2], [1, 2],
                               [2, ncur // 2]])

        def h4_view(lv, c0, ncur):
            """[128, 2(lr), 2(cb), ncur] view of h_t[lv] matching the
            (lr-major, cb-minor) gate-block ordering [xl0 xl1 xr0 xr1]."""
            t = h_t[lv]
            half = max(1, NCOLS[lv] // 2)
            return bass.AP(tensor=t.tensor, offset=t.offset + c0,
                           ap=[list(t.ap[0]), [half, 2], [2 * half, 2],
                               [1, ncur]])

        state = {}   # (lv, ci) -> dict of tiles/views for the back phase

        def emit_leaf_front(ci):
            c0 = ci * NC
            x_c = []
            for kc in range(2):
                t = xpool.tile([128, NC], BF16, tag=f"x{kc}", name=f"x{kc}")
                nc.sync.dma_start(out=t, in_=x[kc, :, c0:c0 + NC])
                x_c.append(t)
            srzx = gates_pool.tile([128, 4, NC], BF16, tag="gr", name="srzx")
            for q in range(2):
                ps = psum.tile([128, 2, NC], F32, tag="ps", name="ps_rzx")
                for mb in range(2):
                    col = 256 + (q * 2 + mb) * 128
                    for kc in range(2):
                        mm(ps[:, mb, :], w_grzx[kc][:, col:col + 128],
                           x_c[kc], start=(kc == 0), stop=(kc == 1))
                if fast_bias:
                    nc.scalar.activation(srzx[:, 2 * q:2 * q + 2, :], ps,
                                         SIGMOID, bias=1.0)
                else:
                    for mb in range(2):
                        nc.scalar.activation(
                            srzx[:, 2 * q + mb, :], ps[:, mb, :],
                            SIGMOID, bias=b_t[2 + 2 * q + mb])
            ps_gx = psum.tile([128, 2, NC], F32, tag="ps", name="ps_gx")
            for mb in range(2):
                for kc in range(2):
                    mm(ps_gx[:, mb, :], w_grzx[kc][:, 128 * mb:128 * mb + 128],
                       x_c[kc], start=(kc == 0), stop=(kc == 1))
            tg = gsb_pool.tile([128, 2, NC], BF16, tag="gsb", name="tg")
            if fast_bias:
                nc.scalar.activation(tg, ps_gx, TANH, bias=0.0)
            else:
                for mb in range(2):
                    nc.scalar.activation(tg[:, mb, :], ps_gx[:, mb, :],
                                         TANH, bias=b_t[mb])
            zsum = scratch.tile([128, 2, NC], BF16, tag="sa", name="zsum")
            nc.gpsimd.tensor_add(zsum, srzx[:, 0:2, :], srzx[:, 2:4, :])
            tt = scratch.tile([128, 2, NC], BF16, tag="sb", name="tt")
            nc.vector.tensor_scalar(tt, zsum, -0.5, 1.0, MULT, ADD)
            j0 = c0 // 2
            nc.vector.tensor_mul(h_t[0][:, :, :, j0:j0 + NC // 2],
                                 parj(tt, NC), parj(tg, NC))
            if ci == nchunks[0] - 1:
                for cb in range(2):
                    nc.sync.dma_start(out=out[cb, :, OFFS[0]:OFFS[1]],
                                      in_=h_t[0][:, cb, :, :])

        def emit_front(lv, ci):
            if lv == 0:
                emit_leaf_front(ci)
                return
            n = NCOLS[lv]
            ncur = min(n, NC)
            c0 = ci * ncur
            h4 = h4_view(lv - 1, c0, ncur)

            # gr blocks [rl0, rl1, rr0, rr1]; gz blocks [zl0, zl1, zr0, zr1]
            gr = gates_pool.tile([128, 4, ncur], BF16, tag="gr", name="gr")
            gz = gates_pool.tile([128, 4, ncur], BF16, tag="gz", name="gz")
            hp = h_t[lv - 1]
            for q in range(4):
                dst = (gr, gz)[q // 2]
                half = (q % 2) * 2
                ps = psum.tile([128, 2, ncur], F32, tag="ps", name="ps_rz")
                for mb in range(2):
                    col = (q * 2 + mb) * 128
                    for kc in range(4):
                        mm(ps[:, mb, :], w_rzh[kc][:, col:col + 128],
                           hp[:, kc % 2, kc // 2, c0:c0 + ncur],
                           start=(kc == 0), stop=(kc == 3))
                if fast_bias:
                    nc.scalar.activation(dst[:, half:half + 2, :], ps,
                                         SIGMOID, bias=1.0)
                else:
                    bi = (2, 4, 2, 4)[q]
                    for mb in range(2):
                        nc.scalar.activation(dst[:, half + mb, :],
                                             ps[:, mb, :],
                                             SIGMOID, bias=b_t[bi + mb])

            # r-path: s = r_l*h_l + r_r*h_r
            p4 = scratch.tile([128, 4, ncur], BF16, tag="sa", name="p4")
            s = scratch.tile([128, 2, ncur], BF16, tag="sc", name="s", bufs=3)
            nc.vector.tensor_mul(p4, gr, h4)
            nc.vector.tensor_add(s, p4[:, 0:2, :], p4[:, 2:4, :])
            # z-path: zh = z_l*h_l + z_r*h_r ; tt = 1 - (z_l + z_r)/2
            z4 = scratch.tile([128, 4, ncur], BF16, tag="sb", name="z4")
            zh = scratch.tile([128, 2, ncur], BF16, tag="sd", name="zh", bufs=3)
            nc.vector.tensor_mul(z4, gz, h4)
            e1 = nc.gpsimd if ncur >= 256 else nc.vector
            e1.tensor_add(zh, z4[:, 0:2, :], z4[:, 2:4, :])
            zs = scratch.tile([128, 2, ncur], BF16, tag="se", name="zs")
            nc.vector.tensor_add(zs, gz[:, 0:2, :], gz[:, 2:4, :])
            tt = gsb_pool.tile([128, 2, ncur], BF16, tag="tt", name="tt", bufs=3)
            nc.vector.tensor_scalar(tt, zs, -0.5, 1.0, MULT, ADD)
            state[(lv, ci)] = dict(s=s, zh=zh, tt=tt, c0=c0, ncur=ncur)

        def emit_back(lv, ci):
            if lv == 0:
                return
            st = state.pop((lv, ci))
            s, zh, tt = st["s"], st["zh"], st["tt"]
            c0, ncur = st["c0"], st["ncur"]

            psg = psum.tile([128, 2, ncur], F32, tag="ps", name="ps_g")
            for mb in range(2):
                for kc in range(2):
                    mm(psg[:, mb, :], w_gh[kc][:, 128 * mb:128 * mb + 128],
                       s[:, kc, :], start=(kc == 0), stop=(kc == 1))
            g_sb = gsb_pool.tile([128, 2, ncur], BF16, tag="gsb", name="g_sb")
            if fast_bias:
                nc.scalar.activation(g_sb, psg, TANH, bias=0.0)
            else:
                for mb in range(2):
                    nc.scalar.activation(g_sb[:, mb, :], psg[:, mb, :],
                                         TANH, bias=b_t[mb])
            tg = scratch.tile([128, 2, ncur], BF16, tag="sg", name="tg")
            nc.vector.tensor_mul(tg, tt, g_sb)
            j0 = c0 // 2
            nc.vector.tensor_add(h_t[lv][:, :, :, j0:j0 + ncur // 2],
                                 parj(tg, ncur), parj(zh, ncur))
            if ci == nchunks[lv] - 1:
                for cb in range(2):
                    nc.sync.dma_start(out=out[cb, :, OFFS[lv]:OFFS[lv + 1]],
                                      in_=h_t[lv][:, cb, :])

        D = 3
        order = _wavefront_order(nchunks, D)

        def parent_list(lv, ci):
            if lv == 0:
                return []
            if nchunks[lv - 1] == 2 * nchunks[lv]:
                return [(lv - 1, 2 * ci), (lv - 1, 2 * ci + 1)]
            return [(lv - 1, pc) for pc in range(nchunks[lv - 1])]

        pending = []
        done = set()

        def pop_back():
            b = pending.pop(0)
            emit_back(*b)
            done.add(b)

        for ch in order:
            if ch is None:
                if pending:
                    pop_back()
                continue
            lv, ci = ch
            for par in parent_list(lv, ci):
                while par not in done:
                    pop_back()
            emit_front(lv, ci)
            pending.append(ch)
            while len(pending) > D:
                pop_back()
        while pending:
            pop_back()

    nc.compile()
    return nc


def _prep_inputs(inputs, Wgrzx, bgrzx, Wrzh, Wgh):
    """Host-side shard + layout prep. Returns (in_maps, fast_bias)."""
    x = np.ascontiguousarray(inputs, dtype=np.float32)
    Wgrzx = np.asarray(Wgrzx, dtype=np.float32)
    bgrzx = np.asarray(bgrzx, dtype=np.float32)
    Wrzh = np.asarray(Wrzh, dtype=np.float32)
    Wgh = np.asarray(Wgh, dtype=np.float32)

    fast_bias = bool(
        np.all(bgrzx[:MEM] == 0.0) and np.all(bgrzx[MEM:] == 1.0))

    wgrzxT = np.ascontiguousarray(
        Wgrzx.T.reshape(2, 128, 768)).astype(ml_dtypes.bfloat16)
    wrzhT = np.ascontiguousarray(
        Wrzh.T.reshape(4, 128, 1024)).astype(ml_dtypes.bfloat16)
    wghT = np.ascontiguousarray(
        Wgh.T.reshape(2, 128, 256)).astype(ml_dtypes.bfloat16)
    bias6 = np.ascontiguousarray(bgrzx.reshape(6, 128, 1))

    in_maps = []
    for c in range(NCORES):
        xc = x[c * BLOC:(c + 1) * BLOC].reshape(N0, IN_DIM)
        xT = np.ascontiguousarray(xc.T).reshape(2, 128, N0).astype(
            ml_dtypes.bfloat16)
        in_maps.append({
            "xT": xT,
            "wrzh": wrzhT,
            "wgrzx": wgrzxT,
            "wgh": wghT,
            "bias6": bias6,
        })
    return in_maps, fast_bias


def _gather(results):
    """results: list of per-core {'out': [2,128,TOT] bf16} -> [B,2L-1,MEM]."""
    outs = []
    for c in range(len(results)):
        fm = np.asarray(results[c]["out"]).astype(np.float32).reshape(MEM, TOT)
        levels = []
        for lv in range(NLEVELS):
            n = NCOLS[lv]
            blk = fm[:, OFFS[lv]:OFFS[lv + 1]]
            nat = np.empty_like(blk)
            nat[:, 0::2] = blk[:, :n // 2]
            nat[:, 1::2] = blk[:, n // 2:]
            k = n // BLOC
            levels.append(nat.reshape(MEM, BLOC, k).transpose(1, 2, 0))
        outs.append(np.concatenate(levels, axis=1))
    return np.ascontiguousarray(
        np.concatenate(outs, axis=0), dtype=np.float32)


def kernel(**inputs):
    in_maps, fast_bias = _prep_inputs(
        inputs["inputs"], inputs["Wgrzx"], inputs["bgrzx"],
        inputs["Wrzh"], inputs["Wgh"])
    nc = build_nc(fast_bias)
    trace = bool(int(os.environ.get("BTGRU_TRACE", "0")))
    res = run_bass_kernel_spmd(
        nc, in_maps, core_ids=list(range(NCORES)), trace=trace)
    LAST_RESULT.clear()
    LAST_RESULT["exec_time_ns"] = res.exec_time_ns
    LAST_RESULT["profile_json"] = res.profile_json
    return _gather(res.results)


# revision 27
# speedup vs baseline: 1.2392x; 1.2195x over previous
"""BinaryTreeGRU Trainium2 kernel (v2: all-bf16 datapath).

Batch of B=64 complete binary trees (L=512 leaves, 1023 nodes each),
data-parallel over trees across 8 NeuronCores (8 trees/core).

Layout: feature-major: every activation tensor lives in SBUF as
[128 partitions, 2 feature-blocks, n_nodes] (mem dim 256 = 2 blocks of 128).
Level l has N_l = 8 * 512 / 2^l node-columns per core. h for each level is
stored deinterleaved ("parity" layout): column 2j+p lives at [cb, p, j], so
the NEXT level's children are two contiguous halves (h_l / h_r views) and
only the single h-write per node is strided.

All activations/gates/h are bf16 (DVE 2x packed mode, half DMA); matmul
accumulation stays fp32 in PSUM. Weights bf16 (FWL); leaf x stays f32r.

Per internal level:
  PE : rzh = Wrzh @ [h_l ; h_r]   (32 MMs/chunk)  +  Wgh @ s (4 MMs)
  ACT: gates = sigmoid(rzh + 1)   g = tanh(Wgh s)
  DVE: P4 = r4*h4, s = P4l+P4r, Z4 = z4*h4, zs = zl+zr,
       tt = 1 - zs/2 (tensor_scalar), tg = tt*g
  GPS: zh = Z4l+Z4r, h = tg + zh (strided parity write)

Host side only reshapes/casts numpy arrays for sharding and gathers.
"""

import os
from contextlib import ExitStack

import ml_dtypes
import numpy as np

import concourse.bass as bass
import concourse.mybir as mybir
import concourse.tile as tile
from concourse import bacc
from concourse.bass_utils import run_bass_kernel_spmd

F32 = mybir.dt.float32
F32R = mybir.dt.float32r
BF16 = mybir.dt.bfloat16
MULT = mybir.AluOpType.mult
ADD = mybir.AluOpType.add
SIGMOID = mybir.ActivationFunctionType.Sigmoid
TANH = mybir.ActivationFunctionType.Tanh
COPY = mybir.ActivationFunctionType.Copy

MEM = 256
IN_DIM = 256
B = 64
L = 512
NCORES = 8
BLOC = B // NCORES            # trees per core
N0 = BLOC * L                 # leaf columns per core = 4096
NLEVELS = 10                  # 4096,2048,...,8 columns
NCOLS = [N0 >> l for l in range(NLEVELS)]
TOT = sum(NCOLS)              # 8184
OFFS = np.cumsum([0] + NCOLS).tolist()
NC = 256                      # node-column chunk

LAST_RESULT = {}


def _wavefront_order(nchunks, d=2):
    """Topological chunk order interleaving levels.

    Chunk (lv, ci)'s parents are (lv-1, 2ci) and (lv-1, 2ci+1) when level
    lv-1 has 2x the chunks; when levels shrink below NC the parent is the
    single previous-level chunk. Child front must come >= parent_pos + d
    (parent back emitted d steps after its front). Returns list of (lv, ci).
    """
    pos = {}
    order = []
    remaining = [(lv, ci) for lv in range(NLEVELS)
                 for ci in range(nchunks[lv])]

    def parents(lv, ci):
        if lv == 0:
            return []
        if nchunks[lv - 1] == 2 * nchunks[lv]:
            return [(lv - 1, 2 * ci), (lv - 1, 2 * ci + 1)]
        return [(lv - 1, pc) for pc in range(nchunks[lv - 1])]

    t = 0
    while remaining:
        ready = []
        for (lv, ci) in remaining:
            ps = parents(lv, ci)
            dd = d if lv > 1 else (d if lv == 1 else 0)
            if all(p in pos and pos[p] + (2 if lv == 1 else dd + 1) <= t
                   for p in ps):
                ready.append((lv, ci))
        if ready:
            ch = max(ready, key=lambda c: (c[0], -c[1]))
            pos[ch] = t
            order.append(ch)
            remaining.remove(ch)
        else:
            order.append(None)   # spacing step (emit only a back)
        t += 1
    return order


def build_nc(fast_bias: bool):
    nc = bacc.Bacc("TRN2", target_bir_lowering=False, debug=False)

    d_x = nc.dram_tensor("xT", [2, 128, N0], BF16, kind="ExternalInput")
    d_wrzh = nc.dram_tensor("wrzh", [4, 128, 1024], BF16, kind="ExternalInput")
    d_wgrzx = nc.dram_tensor("wgrzx", [2, 128, 768], BF16, kind="ExternalInput")
    d_wgh = nc.dram_tensor("wgh", [2, 128, 256], BF16, kind="ExternalInput")
    d_bias = nc.dram_tensor("bias6", [6, 128, 1], F32, kind="ExternalInput")
    d_out = nc.dram_tensor("out", [2, 128, TOT], BF16, kind="ExternalOutput")

    x = d_x.ap()
    wrzh = d_wrzh.ap()
    wgrzx = d_wgrzx.ap()
    wgh = d_wgh.ap()
    bias6 = d_bias.ap()
    out = d_out.ap()

    mm = nc.tensor.matmul
    nchunks = [max(1, NCOLS[lv] // NC) for lv in range(NLEVELS)]

    with tile.TileContext(nc) as tc, ExitStack() as ctx:
        singles = ctx.enter_context(tc.tile_pool(name="singles", bufs=1))
        xpool = ctx.enter_context(tc.tile_pool(name="xpool", bufs=4))
        gates_pool = ctx.enter_context(tc.tile_pool(name="gates", bufs=4))
        gsb_pool = ctx.enter_context(tc.tile_pool(name="gsb", bufs=4))
        scratch = ctx.enter_context(tc.tile_pool(name="scratch", bufs=3))
        psum = ctx.enter_context(tc.tile_pool(name="psum", bufs=8, space="PSUM"))

        # --- load constants ---
        w_rzh = []
        for kc in range(4):
            t = singles.tile([128, 1024], BF16, tag=f"wrzh{kc}", name=f"wrzh{kc}")
            nc.sync.dma_start(out=t, in_=wrzh[kc])
            w_rzh.append(t)
        w_grzx = []
        for kc in range(2):
            t = singles.tile([128, 768], BF16, tag=f"wgrzx{kc}", name=f"wgrzx{kc}")
            nc.sync.dma_start(out=t, in_=wgrzx[kc])
            w_grzx.append(t)
        w_gh = []
        for kc in range(2):
            t = singles.tile([128, 256], BF16, tag=f"wgh{kc}", name=f"wgh{kc}")
            nc.sync.dma_start(out=t, in_=wgh[kc])
            w_gh.append(t)
        b_t = []
        for i in range(6):
            t = singles.tile([128, 1], F32, tag=f"b{i}", name=f"b{i}")
            nc.sync.dma_start(out=t, in_=bias6[i])
            b_t.append(t)
        # b_t: [0]=bg0 [1]=bg1 [2]=bA0 [3]=bA1 [4]=bB0 [5]=bB1

        h_t = [singles.tile([128, 2, 2, max(1, NCOLS[l] // 2)], BF16,
                            tag=f"h{l}", name=f"h{l}", bufs=1)
               for l in range(NLEVELS)]

        def parj(t, ncur):
            """[128, 2(cb), 2(par), ncur/2] strided-read view of a column-
            ordered [128, 2, ncur] tile: element (cb,par,j) = t[cb, 2j+par].
            Lets the parity-deinterleaved h write be CONTIGUOUS (strided
            bf16 writes are pathologically slow: read-modify-write)."""
            return bass.AP(tensor=t.tensor, offset=t.offset,
                           ap=[list(t.ap[0]), [ncur, 2], [1, 2],
                               [2, ncur // 2]])

        def h4_view(lv, c0, ncur):
            """[128, 2(lr), 2(cb), ncur] view of h_t[lv] matching the
            (lr-major, cb-minor) gate-block ordering [xl0 xl1 xr0 xr1]."""
            t = h_t[lv]
            half = max(1, NCOLS[lv] // 2)
            return bass.AP(tensor=t.tensor, offset=t.offset + c0,
                           ap=[list(t.ap[0]), [half, 2], [2 * half, 2],
                               [1, ncur]])

        state = {}   # (lv, ci) -> dict of tiles/views for the back phase

        def emit_leaf_front(ci):
            c0 = ci * NC
            x_c = []
            for kc in range(2):
                t = xpool.tile([128, NC], BF16, tag=f"x{kc}", name=f"x{kc}")
                nc.sync.dma_start(out=t, in_=x[kc, :, c0:c0 + NC])
                x_c.append(t)
            srzx = gates_pool.tile([128, 4, NC], BF16, tag="gr", name="srzx")
            for q in range(2):
                ps = psum.tile([128, 2, NC], F32, tag="ps", name="ps_rzx")
                for mb in range(2):
                    col = 256 + (q * 2 + mb) * 128
                    for kc in range(2):
                        mm(ps[:, mb, :], w_grzx[kc][:, col:col + 128],
                           x_c[kc], start=(kc == 0), stop=(kc == 1))
                if fast_bias:
                    nc.scalar.activation(srzx[:, 2 * q:2 * q + 2, :], ps,
                                         SIGMOID, bias=1.0)
                else:
                    for mb in range(2):
                        nc.scalar.activation(
                            srzx[:, 2 * q + mb, :], ps[:, mb, :],
                            SIGMOID, bias=b_t[2 + 2 * q + mb])
            ps_gx = psum.tile([128, 2, NC], F32, tag="ps", name="ps_gx")
            for mb in range(2):
                for kc in range(2):
                    mm(ps_gx[:, mb, :], w_grzx[kc][:, 128 * mb:128 * mb + 128],
                       x_c[kc], start=(kc == 0), stop=(kc == 1))
            tg = gsb_pool.tile([128, 2, NC], BF16, tag="gsb", name="tg")
            if fast_bias:
                nc.scalar.activation(tg, ps_gx, TANH, bias=0.0)
            else:
                for mb in range(2):
                    nc.scalar.activation(tg[:, mb, :], ps_gx[:, mb, :],
                                         TANH, bias=b_t[mb])
            zsum = scratch.tile([128, 2, NC], BF16, tag="sa", name="zsum")
            nc.gpsimd.tensor_add(zsum, srzx[:, 0:2, :], srzx[:, 2:4, :])
            tt = scratch.tile([128, 2, NC], BF16, tag="sb", name="tt")
            nc.vector.tensor_scalar(tt, zsum, -0.5, 1.0, MULT, ADD)
            j0 = c0 // 2
            nc.vector.tensor_mul(h_t[0][:, :, :, j0:j0 + NC // 2],
                                 parj(tt, NC), parj(tg, NC))
            if ci == nchunks[0] - 1:
                for cb in range(2):
                    nc.sync.dma_start(out=out[cb, :, OFFS[0]:OFFS[1]],
                                      in_=h_t[0][:, cb, :, :])

        def emit_front(lv, ci):
            if lv == 0:
                emit_leaf_front(ci)
                return
            n = NCOLS[lv]
            ncur = min(n, NC)
            c0 = ci * ncur
            h4 = h4_view(lv - 1, c0, ncur)

            # gr blocks [rl0, rl1, rr0, rr1]; gz blocks [zl0, zl1, zr0, zr1]
            gr = gates_pool.tile([128, 4, ncur], BF16, tag="gr", name="gr")
            gz = gates_pool.tile([128, 4, ncur], BF16, tag="gz", name="gz")
            hp = h_t[lv - 1]
            for q in range(4):
                dst = (gr, gz)[q // 2]
                half = (q % 2) * 2
                ps = psum.tile([128, 2, ncur], F32, tag="ps", name="ps_rz")
                for mb in range(2):
                    col = (q * 2 + mb) * 128
                    for kc in range(4):
                        mm(ps[:, mb, :], w_rzh[kc][:, col:col + 128],
                           hp[:, kc % 2, kc // 2, c0:c0 + ncur],
                           start=(kc == 0), stop=(kc == 3))
                if fast_bias:
                    nc.scalar.activation(dst[:, half:half + 2, :], ps,
                                         SIGMOID, bias=1.0)
                else:
                    bi = (2, 4, 2, 4)[q]
                    for mb in range(2):
                        nc.scalar.activation(dst[:, half + mb, :],
                                             ps[:, mb, :],
                                             SIGMOID, bias=b_t[bi + mb])

            # r-path: s = r_l*h_l + r_r*h_r
            p4 = scratch.tile([128, 4, ncur], BF16, tag="sa", name="p4")
            s = scratch.tile([128, 2, ncur], BF16, tag="sc", name="s", bufs=4)
            nc.vector.tensor_mul(p4, gr, h4)
            nc.vector.tensor_add(s, p4[:, 0:2, :], p4[:, 2:4, :])
            # z-path: zh = z_l*h_l + z_r*h_r ; tt = 1 - (z_l + z_r)/2
            z4 = scratch.tile([128, 4, ncur], BF16, tag="sb", name="z4")
            zh = scratch.tile([128, 2, ncur], BF16, tag="sd", name="zh", bufs=4)
            nc.vector.tensor_mul(z4, gz, h4)
            e1 = nc.gpsimd if ncur >= 256 else nc.vector
            e1.tensor_add(zh, z4[:, 0:2, :], z4[:, 2:4, :])
            zs = scratch.tile([128, 2, ncur], BF16, tag="se", name="zs")
            nc.vector.tensor_add(zs, gz[:, 0:2, :], gz[:, 2:4, :])
            tt = gsb_pool.tile([128, 2, ncur], BF16, tag="tt", name="tt", bufs=4)
            nc.vector.tensor_scalar(tt, zs, -0.5, 1.0, MULT, ADD)
            state[(lv, ci)] = dict(s=s, zh=zh, tt=tt, c0=c0, ncur=ncur)

        def emit_back(lv, ci):
            if lv == 0:
                return
            st = state.pop((lv, ci))
            s, zh, tt = st["s"], st["zh"], st["tt"]
            c0, ncur = st["c0"], st["ncur"]

            psg = psum.tile([128, 2, ncur], F32, tag="ps", name="ps_g")
            for mb in range(2):
                for kc in range(2):
                    mm(psg[:, mb, :], w_gh[kc][:, 128 * mb:128 * mb + 128],
                       s[:, kc, :], start=(kc == 0), stop=(kc == 1))
            g_sb = gsb_pool.tile([128, 2, ncur], BF16, tag="gsb", name="g_sb")
            if fast_bias:
                nc.scalar.activation(g_sb, psg, TANH, bias=0.0)
            else:
                for mb in range(2):
                    nc.scalar.activation(g_sb[:, mb, :], psg[:, mb, :],
                                         TANH, bias=b_t[mb])
            tg = scratch.tile([128, 2, ncur], BF16, tag="sg", name="tg")
            nc.vector.tensor_mul(tg, tt, g_sb)
            j0 = c0 // 2
            nc.vector.tensor_add(h_t[lv][:, :, :, j0:j0 + ncur // 2],
                                 parj(tg, ncur), parj(zh, ncur))
            if ci == nchunks[lv] - 1:
                for cb in range(2):
                    nc.sync.dma_start(out=out[cb, :, OFFS[lv]:OFFS[lv + 1]],
                                      in_=h_t[lv][:, cb, :])

        D = 4
        order = _wavefront_order(nchunks, D)

        def parent_list(lv, ci):
            if lv == 0:
                return []
            if nchunks[lv - 1] == 2 * nchunks[lv]:
                return [(lv - 1, 2 * ci), (lv - 1, 2 * ci + 1)]
            return [(lv - 1, pc) for pc in range(nchunks[lv - 1])]

        pending = []
        done = set()

        def pop_back():
            b = pending.pop(0)
            emit_back(*b)
            done.add(b)

        for ch in order:
            if ch is None:
                if pending:
                    pop_back()
                continue
            lv, ci = ch
            for par in parent_list(lv, ci):
                while par not in done:
                    pop_back()
            emit_front(lv, ci)
            pending.append(ch)
            while len(pending) > D:
                pop_back()
        while pending:
            pop_back()

    nc.compile()
    return nc


def _prep_inputs(inputs, Wgrzx, bgrzx, Wrzh, Wgh):
    """Host-side shard + layout prep. Returns (in_maps, fast_bias)."""
    x = np.ascontiguousarray(inputs, dtype=np.float32)
    Wgrzx = np.asarray(Wgrzx, dtype=np.float32)
    bgrzx = np.asarray(bgrzx, dtype=np.float32)
    Wrzh = np.asarray(Wrzh, dtype=np.float32)
    Wgh = np.asarray(Wgh, dtype=np.float32)

    fast_bias = bool(
        np.all(bgrzx[:MEM] == 0.0) and np.all(bgrzx[MEM:] == 1.0))

    wgrzxT = np.ascontiguousarray(
        Wgrzx.T.reshape(2, 128, 768)).astype(ml_dtypes.bfloat16)
    wrzhT = np.ascontiguousarray(
        Wrzh.T.reshape(4, 128, 1024)).astype(ml_dtypes.bfloat16)
    wghT = np.ascontiguousarray(
        Wgh.T.reshape(2, 128, 256)).astype(ml_dtypes.bfloat16)
    bias6 = np.ascontiguousarray(bgrzx.reshape(6, 128, 1))

    in_maps = []
    for c in range(NCORES):
        xc = x[c * BLOC:(c + 1) * BLOC].reshape(N0, IN_DIM)
        xT = np.ascontiguousarray(xc.T).reshape(2, 128, N0).astype(
            ml_dtypes.bfloat16)
        in_maps.append({
            "xT": xT,
            "wrzh": wrzhT,
            "wgrzx": wgrzxT,
            "wgh": wghT,
            "bias6": bias6,
        })
    return in_maps, fast_bias


def _gather(results):
    """results: list of per-core {'out': [2,128,TOT] bf16} -> [B,2L-1,MEM]."""
    outs = []
    for c in range(len(results)):
        fm = np.asarray(results[c]["out"]).astype(np.float32).reshape(MEM, TOT)
        levels = []
        for lv in range(NLEVELS):
            n = NCOLS[lv]
            blk = fm[:, OFFS[lv]:OFFS[lv + 1]]
            nat = np.empty_like(blk)
            nat[:, 0::2] = blk[:, :n // 2]
            nat[:, 1::2] = blk[:, n // 2:]
            k = n // BLOC
            levels.append(nat.reshape(MEM, BLOC, k).transpose(1, 2, 0))
        outs.append(np.concatenate(levels, axis=1))
    return np.ascontiguousarray(
        np.concatenate(outs, axis=0), dtype=np.float32)


def kernel(**inputs):
    in_maps, fast_bias = _prep_inputs(
        inputs["inputs"], inputs["Wgrzx"], inputs["bgrzx"],
        inputs["Wrzh"], inputs["Wgh"])
    nc = build_nc(fast_bias)
    trace = bool(int(os.environ.get("BTGRU_TRACE", "0")))
    res = run_bass_kernel_spmd(
        nc, in_maps, core_ids=list(range(NCORES)), trace=trace)
    LAST_RESULT.clear()
    LAST_RESULT["exec_time_ns"] = res.exec_time_ns
    LAST_RESULT["profile_json"] = res.profile_json
    return _gather(res.results)
